# revision 1
# baseline (speedup 1.0000x reference)
"""NSGT sliced `arrange(cseq, fwd=True)` as a pure-DMA Trainium2 kernel.

Per band [n_slices, n_chan, n_bins, M]: even slice indices are circularly
shifted left by 3*M//4 along the last dim, odd ones by M//4. A circular
shift is two contiguous block copies per row, so the whole op is 16
DRAM->DRAM DMA transfers per core (4 bands x 2 parities x 2 pieces),
sharded 16 slices/core across 8 NeuronCores (pure data parallel).
"""

import numpy as np

import concourse.bass as bass
import concourse.mybir as mybir
from concourse.bass_utils import run_bass_kernel_spmd

N_CORES = 8
BAND_SHAPES = [
    (128, 2, 32, 1024),
    (128, 2, 64, 2048),
    (128, 2, 64, 4096),
    (128, 2, 32, 8192),
]

_NC_CACHE = {}


def build_nc(reps=1):
    """Build the per-core Bass module. `reps` repeats the whole copy program
    (idempotent) so timing runs can amortize dispatch overhead."""
    nc = bass.Bass()
    jobs = {"sp": [], "act": []}
    for i, (n, c, b, m) in enumerate(BAND_SHAPES):
        nloc = n // N_CORES
        x = nc.dram_tensor(f"x{i}", [nloc, c, b, m], mybir.dt.float32,
                           kind="ExternalInput")
        y = nc.dram_tensor(f"y{i}", [nloc, c, b, m], mybir.dt.float32,
                           kind="ExternalOutput")
        for parity, ring in ((0, "sp"), (1, "act")):
            mid = (3 * m // 4) if parity == 0 else (m // 4)
            keep = m - mid
            jobs[ring].append((y[parity::2, :, :, 0:keep],
                               x[parity::2, :, :, mid:m]))
            jobs[ring].append((y[parity::2, :, :, keep:m],
                               x[parity::2, :, :, 0:mid]))

    with (
        nc.Block() as block,
        nc.semaphore("dma_sp") as sem_sp,
        nc.semaphore("dma_act") as sem_act,
    ):
        def make_body(my_jobs, my_sem, other_sem, n_other):
            def body(eng):
                cnt = 0
                for r in range(reps):
                    for out_ap, in_ap in my_jobs:
                        eng.dma_start(out_ap, in_ap).then_inc(my_sem, 16)
                    cnt += 16 * len(my_jobs)
                    eng.wait_ge(my_sem, cnt)
                    if reps > 1:
                        # keep reps lockstep across both rings for timing
                        eng.wait_ge(other_sem, 16 * n_other * (r + 1))
            return body

        block.sync(make_body(jobs["sp"], sem_sp, sem_act, len(jobs["act"])))
        block.scalar(make_body(jobs["act"], sem_act, sem_sp, len(jobs["sp"])))
    return nc


def _get_nc(reps=1):
    if reps not in _NC_CACHE:
        _NC_CACHE[reps] = build_nc(reps)
    return _NC_CACHE[reps]


def run_sharded(bands, reps=1, **run_kwargs):
    """bands: list of 4 full np arrays. Returns (outputs tuple, raw result)."""
    nc = _get_nc(reps)
    nloc = BAND_SHAPES[0][0] // N_CORES
    in_maps = [
        {f"x{i}": np.ascontiguousarray(bands[i][c * nloc:(c + 1) * nloc])
         for i in range(4)}
        for c in range(N_CORES)
    ]
    res = run_bass_kernel_spmd(nc, in_maps, core_ids=list(range(N_CORES)),
                               **run_kwargs)
    outs = tuple(
        np.concatenate([res.results[c][f"y{i}"] for c in range(N_CORES)], axis=0)
        for i in range(4)
    )
    return outs, res


def kernel(band0, band1, band2, band3):
    bands = [np.ascontiguousarray(b, dtype=np.float32)
             for b in (band0, band1, band2, band3)]
    outs, _ = run_sharded(bands)
    return outs
